# revision 7
# baseline (speedup 1.0000x reference)
"""Trainium2 bit-packing kernel (ConsolidateBits).

Input : x (4096, 32768) float32, uniform [0,1).
Output: (4096, 1024) uint32 — every 32 consecutive values along the last
dim packed into one word, bit i = (x > 0.5) at offset i.

Sharding: data-parallel over the batch dim, 512 rows per core, 8 cores.

Per-core pipeline (~64MB DMA-in per core is the roofline, ~190us):
  DMA  : 16 tiles of [128 part x 8192] f32 (4MB contiguous each)
  DVE  : cmp_lo = (x[seg8 0:4] > 0.5) * 1   -> bf16   (TSP, 2x_2p)
         cmp_hi = (x[seg8 4:8] > 0.5) * 16  -> bf16   (TSP, 2x_2p)
         M1 o1   = lo + hi                   in {0,1,16,17}  (TT bf16, 2x_1p)
         M2 o2   = o1[2:4]*4  + o1[0:2]      (STT, 1x)
         M3 byte = o2[1]*2    + o2[0]        (STT, 1x)
         M4 half = byte_odd*256 + byte_even  -> int32 (STT, 1x)
         M5 word = (half_odd << 16) | half_even      (STT int, 1x)
  DMA  : 16 tiles of [128 x 256] i32 out (viewed uint32 on host)
"""

import sys

if "/opt/trn_rl_repo" not in sys.path:
    sys.path.insert(0, "/opt/trn_rl_repo")

import numpy as np

import concourse.bass as bass  # noqa: F401
import concourse.bacc as bacc
import concourse.mybir as mybir
from concourse.tile import TileContext
from concourse.alu_op_type import AluOpType as A
from concourse.bass_utils import run_bass_kernel_spmd

P = 128
N_CORES = 8
ROWS, COLS = 4096, 32768
ROWS_PER_CORE = ROWS // N_CORES   # 512
F = 8192                          # free-dim elements per partition per tile
NTILES = (ROWS_PER_CORE * COLS) // (P * F)  # 16


def build(ntiles: int = NTILES, free: int = F, gp_cmp_tiles: int = 0):
    """gp_cmp_tiles: how many tiles run their compares on GPSIMD instead
    of DVE (load balancing; 0 = all on DVE)."""
    nc = bacc.Bacc("TRN2", target_bir_lowering=False)
    x = nc.dram_tensor(
        "x", [ntiles * P, free], mybir.dt.float32, kind="ExternalInput"
    )
    # int32 throughout the bitvec path (walrus: bitvec ops cannot cast);
    # reinterpreted as uint32 on the host.
    y = nc.dram_tensor(
        "y", [ntiles * P, free // 32], mybir.dt.int32, kind="ExternalOutput"
    )
    xr = x[:, :].rearrange("(t p) f -> t p f", p=P)
    yr = y[:, :].rearrange("(t p) w -> t p w", p=P)

    nseg = free // 8          # seg8 count per partition
    f32, bf16, i32 = mybir.dt.float32, mybir.dt.bfloat16, mybir.dt.int32

    with TileContext(nc) as tc:
        with (
            tc.tile_pool(name="consts", bufs=1) as cpool,
            tc.tile_pool(name="pool", bufs=2) as pool,
        ):
            # Walrus requires bitvec-op scalars to be integer-typed and
            # match src/dst dtype; immediates lower as f32, so keep the
            # shift amount in a per-partition int32 const AP.
            shift16 = cpool.tile([P, 1], i32)
            nc.vector.memset(shift16[:], 16)

            for t in range(ntiles):
                xt = pool.tile([P, free], f32, tag="xt")
                nc.sync.dma_start(xt[:], xr[t])
                xv = xt[:].rearrange("p (s m) -> p s m", m=8)

                cmp_eng = nc.gpsimd if t < gp_cmp_tiles else nc.vector

                # lo = (x>0.5)*1 on seg8[0:4], hi = (x>0.5)*16 on seg8[4:8]
                lo = pool.tile([P, free // 2], bf16, tag="lo")
                hi = pool.tile([P, free // 2], bf16, tag="hi")
                lov = lo[:].rearrange("p (s m) -> p s m", m=4)
                hiv = hi[:].rearrange("p (s m) -> p s m", m=4)
                cmp_eng.tensor_scalar(
                    out=lov, in0=xv[:, :, 0:4], scalar1=0.5, scalar2=None,
                    op0=A.is_gt,
                )
                cmp_eng.tensor_scalar(
                    out=hiv, in0=xv[:, :, 4:8], scalar1=0.5, scalar2=16.0,
                    op0=A.is_gt, op1=A.mult,
                )

                # M1: o1[s,m] = b[8s+m] + 16*b[8s+m+4]   {0,1,16,17} bf16
                o1 = pool.tile([P, free // 2], bf16, tag="o1")
                nc.vector.tensor_tensor(
                    out=o1[:], in0=lo[:], in1=hi[:], op=A.add
                )

                # M2: o2[s,m] = o1[s,m] + 4*o1[s,m+2]    m in [0,2)
                o2 = pool.tile([P, free // 4], f32, tag="o2")
                o1s = o1[:].rearrange("p (s m) -> p s m", m=4)
                o2v = o2[:].rearrange("p (s m) -> p s m", m=2)
                nc.vector.scalar_tensor_tensor(
                    out=o2v, in0=o1s[:, :, 2:4], scalar=4.0, in1=o1s[:, :, 0:2],
                    op0=A.mult, op1=A.add,
                )

                # M3: byte[s] = o2[s,0] + 2*o2[s,1]      0..255
                byt = pool.tile([P, nseg], f32, tag="byt")
                o2s = o2[:].rearrange("p (s m) -> p s m", m=2)
                nc.vector.scalar_tensor_tensor(
                    out=byt[:].rearrange("p (s one) -> p s one", one=1),
                    in0=o2s[:, :, 1:2], scalar=2.0, in1=o2s[:, :, 0:1],
                    op0=A.mult, op1=A.add,
                )

                # M4: half[k] = byte[2k] + 256*byte[2k+1] -> int32 (<=65535)
                half = pool.tile([P, nseg // 2], i32, tag="half")
                bys = byt[:].rearrange("p (k h) -> p k h", h=2)
                nc.vector.scalar_tensor_tensor(
                    out=half[:].rearrange("p (k one) -> p k one", one=1),
                    in0=bys[:, :, 1:2], scalar=256.0, in1=bys[:, :, 0:1],
                    op0=A.mult, op1=A.add,
                )

                # M5: word[w] = (half[2w+1] << 16) | half[2w]
                wt = pool.tile([P, free // 32], i32, tag="wt")
                hs = half[:].rearrange("p (w h) -> p w h", h=2)
                nc.vector.scalar_tensor_tensor(
                    out=wt[:].rearrange("p (w one) -> p w one", one=1),
                    in0=hs[:, :, 1:2], scalar=shift16[:], in1=hs[:, :, 0:1],
                    op0=A.logical_shift_left, op1=A.bitwise_or,
                )

                nc.sync.dma_start(yr[t], wt[:])

    nc.compile()
    return nc


_NC_CACHE = {}


def _get_nc():
    if "nc" not in _NC_CACHE:
        _NC_CACHE["nc"] = build()
    return _NC_CACHE["nc"]


def _shard(x: np.ndarray):
    return [
        np.ascontiguousarray(
            x[i * ROWS_PER_CORE : (i + 1) * ROWS_PER_CORE].reshape(NTILES * P, F)
        )
        for i in range(N_CORES)
    ]


def run(x: np.ndarray, trace: bool = False):
    """Run the SPMD kernel; returns (full_output, BassKernelResults)."""
    nc = _get_nc()
    in_maps = [{"x": s} for s in _shard(x)]
    res = run_bass_kernel_spmd(nc, in_maps, core_ids=list(range(N_CORES)), trace=trace)
    parts = [
        np.asarray(m["y"]).view(np.uint32).reshape(ROWS_PER_CORE, COLS // 32)
        for m in res.results
    ]
    return np.concatenate(parts, axis=0), res


def kernel(x: np.ndarray) -> np.ndarray:
    out, _ = run(np.asarray(x, dtype=np.float32), trace=False)
    return out


# revision 22
# speedup vs baseline: 357.2720x; 357.2720x over previous
"""Trainium2 bit-packing kernel (ConsolidateBits).

Input : x (4096, 32768) float32, uniform [0,1).
Output: (4096, 1024) uint32 — every 32 consecutive values along the last
dim packed into one word, bit i = (x > 0.5) at offset i.

Sharding: data-parallel over the batch dim, 512 rows per core, 8 cores.

Per-core pipeline (~64MB DMA-in per core is the roofline, ~190us):
  DMA  : 16 tiles of [128 part x 8192] f32 (4MB contiguous each)
  DVE  : cmp_lo = (x[seg8 0:4] > 0.5) * 1   -> bf16   (TSP, 2x_2p)
         cmp_hi = (x[seg8 4:8] > 0.5) * 16  -> bf16   (TSP, 2x_2p)
         M1 o1   = lo + hi                   in {0,1,16,17}  (TT bf16, 2x_1p)
         M2 o2   = o1[2:4]*4  + o1[0:2]      (STT, 1x)
         M3 byte = o2[1]*2    + o2[0]        (STT, 1x)
         M4 half = byte_odd*256 + byte_even  -> int32 (STT, 1x)
         M5 word = (half_odd << 16) | half_even      (STT int, 1x)
  DMA  : 16 tiles of [128 x 256] i32 out (viewed uint32 on host)
"""

import sys

if "/opt/trn_rl_repo" not in sys.path:
    sys.path.insert(0, "/opt/trn_rl_repo")

import numpy as np

import concourse.bass as bass  # noqa: F401
import concourse.bacc as bacc
import concourse.mybir as mybir
from concourse.tile import TileContext
from concourse.alu_op_type import AluOpType as A
from concourse.bass_utils import run_bass_kernel_spmd

P = 128
N_CORES = 8
ROWS, COLS = 4096, 32768
ROWS_PER_CORE = ROWS // N_CORES   # 512
F = 8192                          # free-dim elements per partition per tile
NTILES = (ROWS_PER_CORE * COLS) // (P * F)  # 16


def build(ntiles: int = NTILES, free: int = F, gp_cmp_tiles: int = NTILES,
          reps: int = 1, tail_split: int = 4):
    """gp_cmp_tiles: how many tiles run their compares on GPSIMD instead
    of DVE (load balancing; 0 = all on DVE).
    reps: process the whole input `reps` times (benchmarking only —
    lets wall-clock differencing resolve the per-pass kernel time).
    tail_split: split the LAST tile into this many column sub-tiles so the
    serial compute chain after the final DMA is ~tail_split x shorter."""
    nc = bacc.Bacc("TRN2", target_bir_lowering=False)
    x = nc.dram_tensor(
        "x", [ntiles * P, free], mybir.dt.float32, kind="ExternalInput"
    )
    # int32 throughout the bitvec path (walrus: bitvec ops cannot cast);
    # reinterpreted as uint32 on the host.
    y = nc.dram_tensor(
        "y", [ntiles * P, free // 32], mybir.dt.int32, kind="ExternalOutput"
    )
    xr = x[:, :].rearrange("(t p) f -> t p f", p=P)
    yr = y[:, :].rearrange("(t p) w -> t p w", p=P)

    f32, bf16, i32 = mybir.dt.float32, mybir.dt.bfloat16, mybir.dt.int32

    with TileContext(nc) as tc:
        with (
            tc.tile_pool(name="consts", bufs=1) as cpool,
            tc.tile_pool(name="pool", bufs=2) as big_pool,
            tc.tile_pool(name="subpool", bufs=3) as sub_pool,
        ):
            # Walrus requires bitvec-op scalars to be integer-typed and
            # match src/dst dtype; immediates lower as f32, so keep the
            # shift amount in a per-partition int32 const AP.
            shift16 = cpool.tile([P, 1], i32)
            nc.vector.memset(shift16[:], 16)

            ts = max(1, tail_split)
            assert free % (32 * ts) == 0
            work = []
            for t in range(ntiles - 1):
                work.append((t, 0, free))
            if ts >= 4 and free % 16 == 0:
                # descending widths: the last (smallest) piece bounds the
                # serial compute-chain latency after the final DMA lands
                widths = [free * w // 16 for w in (8, 4, 2, 2)]
            else:
                widths = [free // ts] * ts
            col = 0
            for w in widths:
                work.append((ntiles - 1, col, w))
                col += w
            assert col == free
            work = work * reps

            for t, col0, fw in work:
                # sub-tiles (tail split) get their own, deeper pool so the
                # final small DMAs aren't gated on big-tile slot release;
                # half-width pieces still fit the big pool's slots
                pool = big_pool if fw >= free // 2 else sub_pool
                xt = pool.tile([P, fw], f32, tag="xt")
                nc.sync.dma_start(xt[:], xr[t][:, col0 : col0 + fw])
                xv = xt[:].rearrange("p (s m) -> p s m", m=8)

                # Late tiles' compares go to GPSIMD: at stream end the DVE
                # is the tail's critical path, so keep its residual work low.
                cmp_eng = nc.gpsimd if t >= ntiles - gp_cmp_tiles else nc.vector

                # lo = (x>0.5)*1 on seg8[0:4], hi = (x>0.5)*16 on seg8[4:8]
                lo = pool.tile([P, fw // 2], bf16, tag="lo")
                hi = pool.tile([P, fw // 2], bf16, tag="hi")
                lov = lo[:].rearrange("p (s m) -> p s m", m=4)
                hiv = hi[:].rearrange("p (s m) -> p s m", m=4)
                cmp_eng.tensor_scalar(
                    out=lov, in0=xv[:, :, 0:4], scalar1=0.5, scalar2=None,
                    op0=A.is_gt,
                )
                cmp_eng.tensor_scalar(
                    out=hiv, in0=xv[:, :, 4:8], scalar1=0.5, scalar2=16.0,
                    op0=A.is_gt, op1=A.mult,
                )

                # M1: o1[s,m] = b[8s+m] + 16*b[8s+m+4]   {0,1,16,17} bf16
                o1 = pool.tile([P, fw // 2], bf16, tag="o1")
                nc.vector.tensor_tensor(
                    out=o1[:], in0=lo[:], in1=hi[:], op=A.add
                )

                # M2: o2[s,m] = o1[s,m] + 4*o1[s,m+2]    m in [0,2)
                o2 = pool.tile([P, fw // 4], f32, tag="o2")
                o1s = o1[:].rearrange("p (s m) -> p s m", m=4)
                o2v = o2[:].rearrange("p (s m) -> p s m", m=2)
                nc.vector.scalar_tensor_tensor(
                    out=o2v, in0=o1s[:, :, 2:4], scalar=4.0, in1=o1s[:, :, 0:2],
                    op0=A.mult, op1=A.add,
                )

                # M3: byte[s] = o2[s,0] + 2*o2[s,1]      0..255
                byt = pool.tile([P, fw // 8], f32, tag="byt")
                o2s = o2[:].rearrange("p (s m) -> p s m", m=2)
                nc.vector.scalar_tensor_tensor(
                    out=byt[:].rearrange("p (s one) -> p s one", one=1),
                    in0=o2s[:, :, 1:2], scalar=2.0, in1=o2s[:, :, 0:1],
                    op0=A.mult, op1=A.add,
                )

                # M4: half[k] = byte[2k] + 256*byte[2k+1] -> int32 (<=65535)
                half = pool.tile([P, fw // 16], i32, tag="half")
                bys = byt[:].rearrange("p (k h) -> p k h", h=2)
                nc.vector.scalar_tensor_tensor(
                    out=half[:].rearrange("p (k one) -> p k one", one=1),
                    in0=bys[:, :, 1:2], scalar=256.0, in1=bys[:, :, 0:1],
                    op0=A.mult, op1=A.add,
                )

                # M5: word[w] = (half[2w+1] << 16) | half[2w]
                wt = pool.tile([P, fw // 32], i32, tag="wt")
                hs = half[:].rearrange("p (w h) -> p w h", h=2)
                nc.vector.scalar_tensor_tensor(
                    out=wt[:].rearrange("p (w one) -> p w one", one=1),
                    in0=hs[:, :, 1:2], scalar=shift16[:], in1=hs[:, :, 0:1],
                    op0=A.logical_shift_left, op1=A.bitwise_or,
                )

                nc.sync.dma_start(yr[t][:, col0 // 32 : (col0 + fw) // 32], wt[:])

    nc.compile()
    return nc


_NC_CACHE = {}


def _get_nc():
    if "nc" not in _NC_CACHE:
        _NC_CACHE["nc"] = build()
    return _NC_CACHE["nc"]


def _shard(x: np.ndarray):
    return [
        np.ascontiguousarray(
            x[i * ROWS_PER_CORE : (i + 1) * ROWS_PER_CORE].reshape(NTILES * P, F)
        )
        for i in range(N_CORES)
    ]


def run(x: np.ndarray, trace: bool = False):
    """Run the SPMD kernel; returns (full_output, BassKernelResults)."""
    nc = _get_nc()
    in_maps = [{"x": s} for s in _shard(x)]
    res = run_bass_kernel_spmd(nc, in_maps, core_ids=list(range(N_CORES)), trace=trace)
    parts = [
        np.asarray(m["y"]).view(np.uint32).reshape(ROWS_PER_CORE, COLS // 32)
        for m in res.results
    ]
    return np.concatenate(parts, axis=0), res


def kernel(x: np.ndarray) -> np.ndarray:
    out, _ = run(np.asarray(x, dtype=np.float32), trace=False)
    return out
